# revision 1
# baseline (speedup 1.0000x reference)
"""Trainium2 Bass kernel for single-head attention with RoPE.

Problem (per full input): x [256, 200, 1024], wq/wk/wv [128, 1024], wo [1024, 128]
  q/k/v = x @ w*.T ; RoPE on q,k (positions 1..S-1, class token 0 unrotated)
  out = softmax(q k^T / sqrt(128)) v @ wo.T

Strategy: data-parallel over batch across 8 NeuronCores (32 batches/core).
Per core, per block of 2 batches (400 tokens):
  - x is pre-transposed on host to d-major [dc, 128, t] so QKV projections are
    natural matmuls (contraction dim on partitions).
  - q,k computed head-major [128h, t] with de-interleaved RoPE (wq/wk rows are
    permuted host-side so rotate-half form applies to contiguous 64-row halves);
    RoPE = 2 tensor_mul + tensor_sub/tensor_add on DVE using host cos/sin tables.
  - v head-major, then PE-transposed to seq-major [k, h] for the AV matmul.
  - scores computed in BOTH orientations ([q,k] for softmax row-sums via the
    ACT engine's accum_out, [k,q] for the AV matmul) — cheaper than transposing
    the softmax output. No max-subtraction (fp32 exp is safe at these scales).
  - softmax normalization (1/rowsum) is folded into the final PSUM->SBUF copy
    of the output projection as a per-partition activation scale.
"""

import math

import numpy as np

import concourse.bass as bass
import concourse.mybir as mybir
import concourse.tile as tile
from concourse.bass_utils import run_bass_kernel_spmd

B, S, DIM, HD = 256, 200, 1024, 128
BASE = 10000.0
N_CORES = 8
BS = B // N_CORES      # 32 batches per core
BB = 2                 # batches per block
TB = BB * S            # 400 tokens per block
NBLK = BS // BB        # 16 blocks per core
NDC = DIM // 128       # 8 contraction chunks
F32 = mybir.dt.float32
F32R = mybir.dt.float32r
EXP_SCALE = 1.0 / math.sqrt(HD)
# float32r: fp32-layout operands consumed by the PE's single-pass reduced
# precision matmul (~1.8e-4 rel err at K=128; 4x faster than true fp32 for
# moving dim >= 256). Every matmul uses it; score/AV matmuls pad their
# moving dim from S=200 to SP=256 to stay on the fast path.
# q-position (= output row) chunks within one batch: [0:128], [128:200]
QCH = [(0, 128), (128, S - 128)]
# score/AV matmuls run with moving dim padded to 256 (f32r is 4x slower
# below N=256); the pad columns are zeros / never-read garbage
SP = 256


class _TileContextSplitDrain(tile.TileContext):
    """Workaround: this walrus build rejects >2 sem-wait commands on the
    kernel-tail Drain. Emit each needed wait as its own instruction first."""

    def _drain_and_barrier(self, tick_clock, wait_clock):
        nc = self.nc
        fake = mybir.InstNoOp(
            name=nc.get_next_instruction_name(), ins=[], outs=[],
            engine=mybir.EngineType.SP,
        )
        wait_clock.add_sem_waits(
            fake, tile.ScopedClock({None: tick_clock.global_clock})
        )
        waits = list(fake.sync_info.on_wait) if fake.sync_info is not None else []
        assert self.sems is not None
        handles = {h.name: h for h in self.sems.allocated().values()}
        for w in waits:
            nc.sync.wait_ge(handles[w.ant_name], w.wait_value)
        nc.sync.drain()
        nc.all_engine_barrier()
        popped = nc._tile_sem_poison_stack.pop()
        assert popped is self._sem_poison
        nc.clear_and_free_semaphores(list(self.sems.allocated().values()))
        nc.all_engine_barrier()


def _split_excess_waits(nc):
    """This walrus build accepts 1 sem-wait per instruction (2 on
    EventSemaphore). Tile may attach more; hoist the excess onto standalone
    EventSemaphore instructions right before the owner (same engine, so
    in-order issue preserves the wait semantics)."""
    n = 0
    for b in nc.m.functions[0].blocks:
        insts = b.instructions
        out = []
        for i in insts:
            si = i.sync_info
            if si is not None and len(si.on_wait) > 1:
                keep = 2 if isinstance(i, mybir.InstEventSemaphore) else 1
                waits = list(si.on_wait)
                for w in waits[:-keep] if keep < len(waits) else []:
                    n += 1
                    out.append(mybir.InstEventSemaphore(
                        name=f"{i.name}-evw{n}", ins=[], outs=[],
                        engine=i.engine,
                        sync_info=mybir.SyncInfo(on_wait=[w], on_update=[]),
                    ))
                i.sync_info = mybir.SyncInfo(
                    on_wait=waits[-keep:], on_update=list(si.on_update)
                )
            out.append(i)
        b.instructions = out
    return n


def _build_nc():
    nc = bass.Bass("TRN2", target_bir_lowering=False, debug=False)

    xt = nc.dram_tensor("xt", [NBLK, 128, NDC, TB], F32R, kind="ExternalInput").ap()
    wqt = nc.dram_tensor("wqt", [128, NDC, HD], F32R, kind="ExternalInput").ap()
    wkt = nc.dram_tensor("wkt", [128, NDC, HD], F32R, kind="ExternalInput").ap()
    wvt = nc.dram_tensor("wvt", [128, NDC, HD], F32R, kind="ExternalInput").ap()
    wot = nc.dram_tensor("wot", [HD, DIM], F32R, kind="ExternalInput").ap()
    cosf = nc.dram_tensor("cosf", [128, TB], F32, kind="ExternalInput").ap()
    sinf = nc.dram_tensor("sinf", [128, TB], F32, kind="ExternalInput").ap()
    ident = nc.dram_tensor("ident", [128, 128], F32R, kind="ExternalInput").ap()
    p64 = nc.dram_tensor("p64", [128, 128], F32R, kind="ExternalInput").ap()
    out = nc.dram_tensor("out", [BS, S, DIM], F32, kind="ExternalOutput").ap()

    with _TileContextSplitDrain(nc) as tc:
        with (
            tc.tile_pool(name="singles", bufs=1) as singles,
            tc.tile_pool(name="xt", bufs=4) as xt_pool,
            tc.tile_pool(name="qkv_ps", bufs=2, space="PSUM") as qkv_ps,
            tc.tile_pool(name="attn_ps", bufs=3, space="PSUM") as attn_ps,
            tc.tile_pool(name="out_ps", bufs=3, space="PSUM") as out_ps,
            tc.tile_pool(name="ropetmp", bufs=2) as ropetmp,
            tc.tile_pool(name="heads", bufs=3) as heads,
            tc.tile_pool(name="attn_sb", bufs=4) as attn_sb_pool,
            tc.tile_pool(name="stats", bufs=12) as stats,
            tc.tile_pool(name="outsb", bufs=3) as outsb,
        ):
            # ---- one-time loads (wq + first x chunks first: they gate the
            # first projection matmuls; everything else can land behind) ----
            w_sb = {}
            t = singles.tile([128, NDC * HD], F32R, name="wq", tag="wq")
            nc.sync.dma_start(out=t, in_=wqt)
            w_sb["wq"] = t
            xt0_sb = xt_pool.tile([128, NDC * TB], F32R, name="xt", tag="xt")
            for dc in range(NDC):
                nc.sync.dma_start(
                    out=xt0_sb[:, dc * TB:(dc + 1) * TB], in_=xt[0, :, dc, :]
                )
            for name, src in (("wk", wkt), ("wv", wvt)):
                t = singles.tile([128, NDC * HD], F32R, name=name, tag=name)
                nc.sync.dma_start(out=t, in_=src)
                w_sb[name] = t
            p64_sb = singles.tile([128, 128], F32R, name="p64", tag="p64")
            nc.sync.dma_start(out=p64_sb, in_=p64)
            cos_sb = singles.tile([128, TB], F32, name="cosf", tag="cosf")
            nc.sync.dma_start(out=cos_sb, in_=cosf)
            sin_sb = singles.tile([128, TB], F32, name="sinf", tag="sinf")
            nc.sync.dma_start(out=sin_sb, in_=sinf)
            id_sb = singles.tile([128, 128], F32R, name="ident", tag="ident")
            nc.sync.dma_start(out=id_sb, in_=ident)
            wot_sb = singles.tile([HD, DIM], F32R, name="wot", tag="wot")
            nc.sync.dma_start(out=wot_sb, in_=wot)

            for blk in range(NBLK):
                # ---- load x^T for this block ----
                if blk == 0:
                    xt_sb = xt0_sb
                else:
                    xt_sb = xt_pool.tile([128, NDC * TB], F32R, name="xt",
                                         tag="xt")
                    nc.sync.dma_start(out=xt_sb, in_=xt[blk])

                # ---- QKV projections (head-major [128h, TB]) ----
                def proj(wname):
                    ps = qkv_ps.tile([128, TB], F32, name="proj_ps", tag="proj_ps")
                    for dc in range(NDC):
                        nc.tensor.matmul(
                            ps,
                            lhsT=w_sb[wname][:, dc * HD:(dc + 1) * HD],
                            rhs=xt_sb[:, dc * TB:(dc + 1) * TB],
                            start=(dc == 0),
                            stop=(dc == NDC - 1),
                        )
                    return ps

                q_ps = proj("wq")
                k_ps = proj("wk")
                v_ps = proj("wv")

                # ---- RoPE (de-interleaved rotate-half form) ----
                # DVE lanes cannot mix base partitions, so the half-swap
                # swap(q)[p] = q[(p+64)%128] runs on the PE via a permutation
                # matmul (rhs must be SBUF, hence the ACT evacuation first).
                def rope(ps, tag):
                    qsb = ropetmp.tile([128, TB], F32R, name="pre_" + tag,
                                       tag="pre_" + tag)
                    nc.scalar.copy(qsb, ps)
                    sw_ps = qkv_ps.tile([128, TB], F32, name="proj_ps",
                                        tag="proj_ps")
                    nc.tensor.matmul(sw_ps, lhsT=p64_sb, rhs=qsb,
                                     start=True, stop=True)
                    c = ropetmp.tile([128, TB], F32, name="rope_c", tag="rope_c")
                    u = ropetmp.tile([128, TB], F32, name="rope_u", tag="rope_u")
                    h = heads.tile([128, 2 * SP], F32R, name=tag, tag=tag)
                    nc.gpsimd.memset(h[:, TB:].bitcast(F32), 0.0)
                    nc.vector.tensor_mul(c, qsb.bitcast(F32), cos_sb)
                    # sin table is sign-folded ([-sin; +sin]) so one add
                    # completes the rotation
                    nc.vector.tensor_mul(u, sw_ps, sin_sb)
                    nc.vector.tensor_add(h[:, 0:TB], c, u)
                    return h

                q_h = rope(q_ps, "q_h")
                k_h = rope(k_ps, "k_h")
                v_h = heads.tile([128, TB], F32R, name="v_h", tag="v_h")
                nc.scalar.copy(v_h, v_ps)

                # ---- scores, both orientations, per batch ----
                # s[qc]: [q, k] for row sums; st[kc]: [k, q] for the AV matmul
                s_tiles = []
                for qc, (q0, qsz) in enumerate(QCH):
                    sp = attn_ps.tile([128, 2 * SP], F32, name="attn_ps",
                                      tag="attn_ps")
                    for i in range(BB):
                        nc.tensor.matmul(
                            sp[0:qsz, i * SP:(i + 1) * SP],
                            lhsT=q_h[:, i * S + q0: i * S + q0 + qsz],
                            rhs=k_h[:, i * S: i * S + SP],
                            start=True, stop=True,
                        )
                    s_tiles.append(sp)

                # exp row sums via ACT accum_out (per batch, per q-chunk)
                recips = {}
                scratch = [
                    attn_sb_pool.tile([128, 2 * SP], F32, name="exp_scr", tag="exp_scr")
                    for _ in QCH
                ]
                for qc, (q0, qsz) in enumerate(QCH):
                    for i in range(BB):
                        sums = stats.tile([128, 1], F32, name="sums", tag="sums")
                        nc.scalar.activation(
                            out=scratch[qc][0:qsz, i * SP: i * SP + S],
                            in_=s_tiles[qc][0:qsz, i * SP: i * SP + S],
                            func=mybir.ActivationFunctionType.Exp,
                            scale=EXP_SCALE,
                            accum_out=sums[0:qsz, :],
                        )
                        rec = stats.tile([128, 1], F32, name="recip", tag="recip")
                        nc.vector.reciprocal(rec[0:qsz, :], sums[0:qsz, :])
                        recips[(i, qc)] = rec

                est = []
                for kc, (k0, ksz) in enumerate(QCH):
                    stp = attn_ps.tile([128, 2 * SP], F32, name="attn_ps",
                                       tag="attn_ps")
                    for i in range(BB):
                        nc.tensor.matmul(
                            stp[0:ksz, i * SP:(i + 1) * SP],
                            lhsT=k_h[:, i * S + k0: i * S + k0 + ksz],
                            rhs=q_h[:, i * S: i * S + SP],
                            start=True, stop=True,
                        )
                    e = attn_sb_pool.tile([128, 2 * SP], F32R, name="exp_st",
                                          tag="exp_st")
                    nc.scalar.activation(
                        out=e[0:ksz, :], in_=stp[0:ksz, :],
                        func=mybir.ActivationFunctionType.Exp,
                        scale=EXP_SCALE,
                    )
                    est.append(e)

                # ---- V -> seq-major via PE transpose, per batch ----
                vt_sbs = []
                for i in range(BB):
                    vt_ps = attn_ps.tile([128, 256], F32R, name="vt_ps", tag="attn_ps")
                    nc.tensor.transpose(
                        vt_ps[0:128, 0:128],
                        v_h[:, i * S: i * S + 128], id_sb,
                    )
                    nc.tensor.transpose(
                        vt_ps[0:72, 128:256],
                        v_h[:, i * S + 128: (i + 1) * S], id_sb,
                    )
                    vt_sb = attn_sb_pool.tile([128, 256], F32R, name="vt_sb", tag="vt_sb")
                    nc.vector.tensor_copy(
                        vt_sb[0:128, 0:128], vt_ps[0:128, 0:128]
                    )
                    nc.vector.tensor_copy(
                        vt_sb[0:72, 128:256], vt_ps[0:72, 128:256]
                    )
                    vt_sbs.append(vt_sb)

                # ---- AV: attn_head[h, q] (unnormalized) ----
                av_ps = attn_ps.tile([128, 2 * SP], F32, name="attn_ps",
                                     tag="attn_ps")
                for i in range(BB):
                    nc.tensor.matmul(
                        av_ps[:, i * SP:(i + 1) * SP],
                        lhsT=vt_sbs[i][0:128, 0:128],
                        rhs=est[0][0:128, i * SP:(i + 1) * SP],
                        start=True, stop=False,
                    )
                    nc.tensor.matmul(
                        av_ps[:, i * SP:(i + 1) * SP],
                        lhsT=vt_sbs[i][0:72, 128:256],
                        rhs=est[1][0:72, i * SP:(i + 1) * SP],
                        start=False, stop=True,
                    )
                av_sb = attn_sb_pool.tile([128, 2 * SP], F32R, name="av_sb",
                                          tag="av_sb")
                for i in range(BB):
                    nc.vector.tensor_copy(
                        av_sb[:, i * SP:(i + 1) * SP],
                        av_ps[:, i * SP:(i + 1) * SP],
                    )

                # ---- output projection + normalization + store ----
                osb = {
                    (i, tc_i): outsb.tile([128, DIM], F32,
                                          name=f"osb{i}_{tc_i}",
                                          tag=f"osb{i}_{tc_i}")
                    for i in range(BB) for tc_i in range(len(QCH))
                }
                for i in range(BB):
                    for tc_i, (t0, tsz) in enumerate(QCH):
                        for dc in range(2):
                            ops = out_ps.tile([128, 512], F32, name="out_ps", tag="out_ps")
                            nc.tensor.matmul(
                                ops[0:tsz, :],
                                lhsT=av_sb[:, i * SP + t0: i * SP + t0 + tsz],
                                rhs=wot_sb[:, dc * 512:(dc + 1) * 512],
                                start=True, stop=True,
                            )
                            dst = osb[(i, tc_i)][0:tsz, dc * 512:
                                                 (dc + 1) * 512]
                            if dc == 0:
                                nc.scalar.mul(dst, ops[0:tsz, :],
                                              recips[(i, tc_i)][0:tsz, :])
                            else:
                                nc.vector.tensor_scalar_mul(
                                    dst, ops[0:tsz, :],
                                    recips[(i, tc_i)][0:tsz, :])
                for i in range(BB):
                    for tc_i, (t0, tsz) in enumerate(QCH):
                        nc.sync.dma_start(
                            out=out[blk * BB + i, t0:t0 + tsz, :],
                            in_=osb[(i, tc_i)][0:tsz, :],
                        )
    _split_excess_waits(nc)
    return nc


_NC_CACHE = {}


def _get_nc():
    if "nc" not in _NC_CACHE:
        _NC_CACHE["nc"] = _build_nc()
    return _NC_CACHE["nc"]


def _host_prep(x, wq, wk, wv, wo):
    """Shared (non-x) device inputs + per-core x^T shards."""
    perm = np.concatenate([np.arange(0, HD, 2), np.arange(1, HD, 2)])
    # weight layout [p, dc, h]: row d of w.T at (p=d%128, dc=d//128)
    wqt = np.ascontiguousarray(
        wq[perm].T.reshape(NDC, 128, HD).transpose(1, 0, 2))
    wkt = np.ascontiguousarray(
        wk[perm].T.reshape(NDC, 128, HD).transpose(1, 0, 2))
    wvt = np.ascontiguousarray(
        wv.T.reshape(NDC, 128, HD).transpose(1, 0, 2))
    wot = np.ascontiguousarray(wo.T)

    inv_freq = 1.0 / BASE ** (np.arange(0, HD, 2, dtype=np.float64) / HD)
    ang = np.zeros((S, HD // 2), np.float64)
    ang[1:] = np.arange(S - 1, dtype=np.float64)[:, None] * inv_freq[None, :]
    cos_t = np.cos(ang).T.astype(np.float32)   # [64, S]
    sin_t = np.sin(ang).T.astype(np.float32)
    cosf = np.tile(np.concatenate([cos_t, cos_t], axis=0), (1, BB))  # [128, TB]
    # sign-folded: rotated = q*cosf + swap64(q)*sinf in one add
    sinf = np.tile(np.concatenate([-sin_t, sin_t], axis=0), (1, BB))
    ident = np.eye(128, dtype=np.float32)

    shared = {
        "wqt": wqt, "wkt": wkt, "wvt": wvt, "wot": wot,
        "cosf": np.ascontiguousarray(cosf),
        "sinf": np.ascontiguousarray(sinf),
        "ident": ident,
        "p64": np.ascontiguousarray(
            np.roll(np.eye(128, dtype=np.float32), 64, axis=1)
        ),
    }
    xts = []
    for c in range(N_CORES):
        xc = x[c * BS:(c + 1) * BS].reshape(NBLK, TB, NDC, 128)
        # [blk, p, dc, t]: one contiguous 12.8KB run per partition per DMA
        xt = np.ascontiguousarray(xc.transpose(0, 3, 2, 1))
        xts.append(xt)
    return shared, xts


def kernel(x, wq, wk, wv, wo):
    x = np.asarray(x, np.float32)
    wq = np.asarray(wq, np.float32)
    wk = np.asarray(wk, np.float32)
    wv = np.asarray(wv, np.float32)
    wo = np.asarray(wo, np.float32)

    shared, xts = _host_prep(x, wq, wk, wv, wo)
    in_maps = [dict(shared, xt=xts[c]) for c in range(N_CORES)]
    nc = _get_nc()
    res = run_bass_kernel_spmd(nc, in_maps, list(range(N_CORES)))
    return np.concatenate(
        [res.results[c]["out"] for c in range(N_CORES)], axis=0
    ).reshape(B, S, DIM)



# revision 2
# speedup vs baseline: 1.2469x; 1.2469x over previous
"""Trainium2 Bass kernel for single-head attention with RoPE.

Problem (per full input): x [256, 200, 1024], wq/wk/wv [128, 1024], wo [1024, 128]
  q/k/v = x @ w*.T ; RoPE on q,k (positions 1..S-1, class token 0 unrotated)
  out = softmax(q k^T / sqrt(128)) v @ wo.T

Strategy: data-parallel over batch across 8 NeuronCores (32 batches/core).
All data-plane traffic and matmuls run in bf16 (fp32 PSUM accumulation):
  - x is pre-transposed AND pre-cast to bf16 on host -> input DMA halves.
  - output is written bf16 and upcast on host -> output DMA halves.
  - bf16 matmuls cost 1 PE cycle/row with no 256-wide moving-dim
    constraint (fp32r needed padding to 256), so score/AV matmuls run
    at their natural 200-token width.
Per core, per block of 2 batches (400 tokens):
  - QKV projections head-major [128h, t] from d-major x^T chunks.
  - RoPE: u = q*sin' and c = q*cos on DVE straight from PSUM; the
    half-swap swap64(u) runs as an SBUF->SBUF DMA partition rotation
    (2 64-partition copies for q+k together); h = c + sw on the
    otherwise-idle Pool engine (SBUF-only operands).
  - scores in BOTH orientations ([q,k] for ACT-accumulated row sums,
    [k,q] for the AV matmul); exp on ACT; no max-subtraction.
  - V transposed seq-major via PE-transpose (bf16: 1 cycle/row).
  - softmax normalization (1/rowsum) folded into the PSUM->SBUF copy of
    the output projection as a per-partition scale (ACT/DVE split).
"""

import math

import numpy as np
import ml_dtypes

import concourse.bass as bass
import concourse.mybir as mybir
import concourse.tile as tile
from concourse.bass_utils import run_bass_kernel_spmd

B, S, DIM, HD = 256, 200, 1024, 128
BASE = 10000.0
N_CORES = 8
BS = B // N_CORES      # 32 batches per core
BB = 2                 # batches per block
TB = BB * S            # 400 tokens per block
NBLK = BS // BB        # 16 blocks per core
NDC = DIM // 128       # 8 contraction chunks
F32 = mybir.dt.float32
BF16 = mybir.dt.bfloat16
EXP_SCALE = 1.0 / math.sqrt(HD)
# q-position (= output row) chunks within one batch: [0:128], [128:200]
QCH = [(0, 128), (128, S - 128)]


class _TileContextSplitDrain(tile.TileContext):
    """Workaround: this walrus build rejects >2 sem-wait commands on the
    kernel-tail Drain. Emit each needed wait as its own instruction first."""

    def _drain_and_barrier(self, tick_clock, wait_clock):
        nc = self.nc
        fake = mybir.InstNoOp(
            name=nc.get_next_instruction_name(), ins=[], outs=[],
            engine=mybir.EngineType.SP,
        )
        wait_clock.add_sem_waits(
            fake, tile.ScopedClock({None: tick_clock.global_clock})
        )
        waits = list(fake.sync_info.on_wait) if fake.sync_info is not None else []
        assert self.sems is not None
        handles = {h.name: h for h in self.sems.allocated().values()}
        for w in waits:
            nc.sync.wait_ge(handles[w.ant_name], w.wait_value)
        nc.sync.drain()
        nc.all_engine_barrier()
        popped = nc._tile_sem_poison_stack.pop()
        assert popped is self._sem_poison
        nc.clear_and_free_semaphores(list(self.sems.allocated().values()))
        nc.all_engine_barrier()


def _split_excess_waits(nc):
    """This walrus build accepts 1 sem-wait per instruction (2 on
    EventSemaphore). Tile may attach more; hoist the excess onto standalone
    EventSemaphore instructions right before the owner (same engine, so
    in-order issue preserves the wait semantics)."""
    n = 0
    for b in nc.m.functions[0].blocks:
        insts = b.instructions
        out = []
        for i in insts:
            si = i.sync_info
            if si is not None and len(si.on_wait) > 1:
                keep = 2 if isinstance(i, mybir.InstEventSemaphore) else 1
                waits = list(si.on_wait)
                for w in waits[:-keep] if keep < len(waits) else []:
                    n += 1
                    out.append(mybir.InstEventSemaphore(
                        name=f"{i.name}-evw{n}", ins=[], outs=[],
                        engine=i.engine,
                        sync_info=mybir.SyncInfo(on_wait=[w], on_update=[]),
                    ))
                i.sync_info = mybir.SyncInfo(
                    on_wait=waits[-keep:], on_update=list(si.on_update)
                )
            out.append(i)
        b.instructions = out
    return n


def _build_nc():
    nc = bass.Bass("TRN2", target_bir_lowering=False, debug=False)

    xt = nc.dram_tensor("xt", [NBLK, 128, NDC * TB], BF16, kind="ExternalInput").ap()
    wqt = nc.dram_tensor("wqt", [128, NDC * HD], BF16, kind="ExternalInput").ap()
    wkt = nc.dram_tensor("wkt", [128, NDC * HD], BF16, kind="ExternalInput").ap()
    wvt = nc.dram_tensor("wvt", [128, NDC * HD], BF16, kind="ExternalInput").ap()
    wot = nc.dram_tensor("wot", [HD, DIM], BF16, kind="ExternalInput").ap()
    cosf = nc.dram_tensor("cosf", [128, TB], F32, kind="ExternalInput").ap()
    s2f = nc.dram_tensor("s2f", [128, TB], F32, kind="ExternalInput").ap()
    ident = nc.dram_tensor("ident", [128, 128], BF16, kind="ExternalInput").ap()
    out = nc.dram_tensor("out", [BS, S, DIM], BF16, kind="ExternalOutput").ap()

    with _TileContextSplitDrain(nc) as tc:
        with (
            tc.tile_pool(name="singles", bufs=1) as singles,
            tc.tile_pool(name="xt", bufs=3) as xt_pool,
            tc.tile_pool(name="qkv_ps", bufs=2, space="PSUM") as qkv_ps,
            tc.tile_pool(name="attn_ps", bufs=4, space="PSUM") as attn_ps,
            tc.tile_pool(name="out_ps", bufs=2, space="PSUM") as out_ps,
            tc.tile_pool(name="ropetmp", bufs=2) as ropetmp,
            tc.tile_pool(name="heads", bufs=3) as heads,
            tc.tile_pool(name="attn_sb", bufs=4) as attn_sb_pool,
            tc.tile_pool(name="stats", bufs=8) as stats,
            tc.tile_pool(name="outsb", bufs=3) as outsb,
        ):
            # ---- one-time loads (wq + first x first: they gate the first
            # projection matmuls; everything else can land behind) ----
            w_sb = {}
            t = singles.tile([128, NDC * HD], BF16, name="wq", tag="wq")
            nc.sync.dma_start(out=t, in_=wqt)
            w_sb["wq"] = t
            xt0_sb = xt_pool.tile([128, NDC * TB], BF16, name="xt", tag="xt")
            nc.sync.dma_start(out=xt0_sb, in_=xt[0])
            for name, src in (("wk", wkt), ("wv", wvt)):
                t = singles.tile([128, NDC * HD], BF16, name=name, tag=name)
                nc.sync.dma_start(out=t, in_=src)
                w_sb[name] = t
            cos_sb = singles.tile([128, TB], F32, name="cosf", tag="cosf")
            nc.sync.dma_start(out=cos_sb, in_=cosf)
            s2_sb = singles.tile([128, TB], F32, name="s2f", tag="s2f")
            nc.sync.dma_start(out=s2_sb, in_=s2f)
            id_sb = singles.tile([128, 128], BF16, name="ident", tag="ident")
            nc.sync.dma_start(out=id_sb, in_=ident)
            wot_sb = singles.tile([HD, DIM], BF16, name="wot", tag="wot")
            nc.sync.dma_start(out=wot_sb, in_=wot)

            for blk in range(NBLK):
                # ---- load x^T for this block (one DMA, 6.4KB/partition) ----
                if blk == 0:
                    xt_sb = xt0_sb
                else:
                    xt_sb = xt_pool.tile([128, NDC * TB], BF16, name="xt",
                                         tag="xt")
                    nc.sync.dma_start(out=xt_sb, in_=xt[blk])

                # ---- QKV projections (head-major [128h, TB]) ----
                def proj(wname):
                    ps = qkv_ps.tile([128, TB], F32, name="proj_ps", tag="proj_ps")
                    for dc in range(NDC):
                        nc.tensor.matmul(
                            ps,
                            lhsT=w_sb[wname][:, dc * HD:(dc + 1) * HD],
                            rhs=xt_sb[:, dc * TB:(dc + 1) * TB],
                            start=(dc == 0),
                            stop=(dc == NDC - 1),
                        )
                    return ps

                q_ps = proj("wq")
                k_ps = proj("wk")
                v_ps = proj("wv")

                # ---- RoPE (de-interleaved rotate-half form) ----
                # rotated = q*cosf + swap64(q)*sinf, where sinf is
                # sign-folded. With s2 = swap64(sinf): swap64(q)*sinf =
                # swap64(q*s2), and the partition half-swap runs as an
                # SBUF->SBUF DMA (q and k in one tile -> 2 DMAs total).
                u2 = ropetmp.tile([128, 2 * TB], BF16, name="u2", tag="u2")
                c2 = ropetmp.tile([128, 2 * TB], BF16, name="c2", tag="c2")
                sw2 = ropetmp.tile([128, 2 * TB], BF16, name="sw2", tag="sw2")
                for col, ps in ((0, q_ps), (TB, k_ps)):
                    nc.vector.tensor_mul(u2[:, col:col + TB], ps, s2_sb)
                    nc.vector.tensor_mul(c2[:, col:col + TB], ps, cos_sb)
                nc.sync.dma_start(out=sw2[0:64, :], in_=u2[64:128, :])
                nc.sync.dma_start(out=sw2[64:128, :], in_=u2[0:64, :])
                q_h = heads.tile([128, TB], BF16, name="q_h", tag="q_h")
                k_h = heads.tile([128, TB], BF16, name="k_h", tag="k_h")
                for col, h in ((0, q_h), (TB, k_h)):
                    nc.gpsimd.tensor_add(h, c2[:, col:col + TB],
                                         sw2[:, col:col + TB])
                v_h = heads.tile([128, TB], BF16, name="v_h", tag="v_h")
                nc.scalar.copy(v_h, v_ps)

                # ---- V -> seq-major via PE transpose, per batch ----
                vt_sbs = []
                for i in range(BB):
                    vt_ps = attn_ps.tile([128, 256], BF16, name="vt_ps",
                                         tag="attn_ps")
                    nc.tensor.transpose(
                        vt_ps[0:128, 0:128],
                        v_h[:, i * S: i * S + 128], id_sb,
                    )
                    nc.tensor.transpose(
                        vt_ps[0:72, 128:256],
                        v_h[:, i * S + 128: (i + 1) * S], id_sb,
                    )
                    vt_sb = attn_sb_pool.tile([128, 256], BF16, name="vt_sb",
                                              tag="vt_sb")
                    nc.vector.tensor_copy(vt_sb[0:128, 0:128],
                                          vt_ps[0:128, 0:128])
                    nc.vector.tensor_copy(vt_sb[0:72, 128:256],
                                          vt_ps[0:72, 128:256])
                    vt_sbs.append(vt_sb)

                # ---- scores [q,k] -> exp row-sums via ACT accum_out ----
                recips = {}
                for qc, (q0, qsz) in enumerate(QCH):
                    sp = attn_ps.tile([128, TB], F32, name="s_ps",
                                      tag="attn_ps")
                    for i in range(BB):
                        nc.tensor.matmul(
                            sp[0:qsz, i * S:(i + 1) * S],
                            lhsT=q_h[:, i * S + q0: i * S + q0 + qsz],
                            rhs=k_h[:, i * S:(i + 1) * S],
                            start=True, stop=True,
                        )
                    scratch = attn_sb_pool.tile([128, TB], BF16,
                                                name="exp_scr", tag="exp_scr")
                    sums = stats.tile([128, BB], F32, name="sums", tag="sums")
                    for i in range(BB):
                        nc.scalar.activation(
                            out=scratch[0:qsz, i * S:(i + 1) * S],
                            in_=sp[0:qsz, i * S:(i + 1) * S],
                            func=mybir.ActivationFunctionType.Exp,
                            scale=EXP_SCALE,
                            accum_out=sums[0:qsz, i:i + 1],
                        )
                    rec = stats.tile([128, BB], F32, name="recip", tag="recip")
                    nc.vector.reciprocal(rec[0:qsz, :], sums[0:qsz, :])
                    recips[qc] = rec

                # ---- scores [k,q] -> exp'd (unnormalized probs for AV) ----
                est = []
                for kc, (k0, ksz) in enumerate(QCH):
                    stp = attn_ps.tile([128, TB], F32, name="st_ps",
                                       tag="attn_ps")
                    for i in range(BB):
                        nc.tensor.matmul(
                            stp[0:ksz, i * S:(i + 1) * S],
                            lhsT=k_h[:, i * S + k0: i * S + k0 + ksz],
                            rhs=q_h[:, i * S:(i + 1) * S],
                            start=True, stop=True,
                        )
                    e = attn_sb_pool.tile([128, TB], BF16, name="exp_st",
                                          tag="exp_st")
                    nc.scalar.activation(
                        out=e[0:ksz, :], in_=stp[0:ksz, :],
                        func=mybir.ActivationFunctionType.Exp,
                        scale=EXP_SCALE,
                    )
                    est.append(e)

                # ---- AV: attn_head[h, t] (unnormalized) ----
                av_ps = attn_ps.tile([128, TB], F32, name="av_ps",
                                     tag="attn_ps")
                for i in range(BB):
                    nc.tensor.matmul(
                        av_ps[:, i * S:(i + 1) * S],
                        lhsT=vt_sbs[i][0:128, 0:128],
                        rhs=est[0][0:128, i * S:(i + 1) * S],
                        start=True, stop=False,
                    )
                    nc.tensor.matmul(
                        av_ps[:, i * S:(i + 1) * S],
                        lhsT=vt_sbs[i][0:72, 128:256],
                        rhs=est[1][0:72, i * S:(i + 1) * S],
                        start=False, stop=True,
                    )
                av_sb = attn_sb_pool.tile([128, TB], BF16, name="av_sb",
                                          tag="av_sb")
                nc.vector.tensor_copy(av_sb, av_ps)

                # ---- output projection + normalization + store ----
                for i in range(BB):
                    for tc_i, (t0, tsz) in enumerate(QCH):
                        osb = outsb.tile([128, DIM], BF16,
                                         name=f"osb{i}_{tc_i}",
                                         tag=f"osb{i}_{tc_i}")
                        for dc in range(2):
                            ops = out_ps.tile([128, 512], F32, name="out_ps",
                                              tag="out_ps")
                            nc.tensor.matmul(
                                ops[0:tsz, :],
                                lhsT=av_sb[:, i * S + t0: i * S + t0 + tsz],
                                rhs=wot_sb[:, dc * 512:(dc + 1) * 512],
                                start=True, stop=True,
                            )
                            dst = osb[0:tsz, dc * 512:(dc + 1) * 512]
                            if dc == 0:
                                nc.scalar.mul(dst, ops[0:tsz, :],
                                              recips[tc_i][0:tsz, i:i + 1])
                            else:
                                nc.vector.tensor_scalar_mul(
                                    dst, ops[0:tsz, :],
                                    recips[tc_i][0:tsz, i:i + 1])
                        nc.sync.dma_start(
                            out=out[blk * BB + i, t0:t0 + tsz, :],
                            in_=osb[0:tsz, :],
                        )
    _split_excess_waits(nc)
    return nc


_NC_CACHE = {}


def _get_nc():
    if "nc" not in _NC_CACHE:
        _NC_CACHE["nc"] = _build_nc()
    return _NC_CACHE["nc"]


def _host_prep(x, wq, wk, wv, wo):
    """Shared (non-x) device inputs + per-core x^T shards (all bf16)."""
    perm = np.concatenate([np.arange(0, HD, 2), np.arange(1, HD, 2)])
    # weight layout [p, dc*HD + h]: row d of w.T at (p=d%128, dc=d//128)
    def wlay(w):
        return np.ascontiguousarray(
            w.T.reshape(NDC, 128, HD).transpose(1, 0, 2).reshape(128, NDC * HD)
        ).astype(ml_dtypes.bfloat16)

    wqt = wlay(wq[perm])
    wkt = wlay(wk[perm])
    wvt = wlay(wv)
    wot = np.ascontiguousarray(wo.T).astype(ml_dtypes.bfloat16)

    inv_freq = 1.0 / BASE ** (np.arange(0, HD, 2, dtype=np.float64) / HD)
    ang = np.zeros((S, HD // 2), np.float64)
    ang[1:] = np.arange(S - 1, dtype=np.float64)[:, None] * inv_freq[None, :]
    cos_t = np.cos(ang).T.astype(np.float32)   # [64, S]
    sin_t = np.sin(ang).T.astype(np.float32)
    cosf = np.tile(np.concatenate([cos_t, cos_t], axis=0), (1, BB))  # [128, TB]
    # sign-folded: rotated = q*cosf + swap64(q)*sinf in one add
    sinf = np.tile(np.concatenate([-sin_t, sin_t], axis=0), (1, BB))
    # s2 = swap64(sinf) so that swap64(q*s2) == swap64(q)*sinf
    s2f = np.roll(sinf, -64, axis=0)
    ident = np.eye(128, dtype=ml_dtypes.bfloat16)

    shared = {
        "wqt": wqt, "wkt": wkt, "wvt": wvt, "wot": wot,
        "cosf": np.ascontiguousarray(cosf),
        "s2f": np.ascontiguousarray(s2f),
        "ident": ident,
    }
    xts = []
    for c in range(N_CORES):
        xc = x[c * BS:(c + 1) * BS].reshape(NBLK, TB, NDC, 128)
        # [blk, p, dc*TB + t]: one contiguous 6.4KB run per partition per DMA
        xtc = np.ascontiguousarray(
            xc.transpose(0, 3, 2, 1).reshape(NBLK, 128, NDC * TB)
        ).astype(ml_dtypes.bfloat16)
        xts.append(xtc)
    return shared, xts


def kernel(x, wq, wk, wv, wo):
    x = np.asarray(x, np.float32)
    wq = np.asarray(wq, np.float32)
    wk = np.asarray(wk, np.float32)
    wv = np.asarray(wv, np.float32)
    wo = np.asarray(wo, np.float32)

    shared, xts = _host_prep(x, wq, wk, wv, wo)
    in_maps = [dict(shared, xt=xts[c]) for c in range(N_CORES)]
    nc = _get_nc()
    res = run_bass_kernel_spmd(nc, in_maps, list(range(N_CORES)))
    return np.concatenate(
        [np.asarray(res.results[c]["out"]).astype(np.float32)
         for c in range(N_CORES)], axis=0
    ).reshape(B, S, DIM)


# revision 3
# speedup vs baseline: 1.4358x; 1.1515x over previous
"""Trainium2 Bass kernel for single-head attention with RoPE.

Problem (per full input): x [256, 200, 1024], wq/wk/wv [128, 1024], wo [1024, 128]
  q/k/v = x @ w*.T ; RoPE on q,k (positions 1..S-1, class token 0 unrotated)
  out = softmax(q k^T / sqrt(128)) v @ wo.T

Strategy: data-parallel over batch across 8 NeuronCores (32 batches/core).
All data-plane traffic and matmuls run in bf16 (fp32 PSUM accumulation):
  - x is pre-transposed AND pre-cast to bf16 on host -> input DMA halves.
  - output is written bf16 and upcast on host -> output DMA halves.
  - bf16 matmuls cost 1 PE cycle/row with no 256-wide moving-dim
    constraint (fp32r needed padding to 256), so score/AV matmuls run
    at their natural 200-token width.
Per core, per block of 2 batches (400 tokens):
  - QKV projections head-major [128h, t] from d-major x^T chunks.
  - RoPE: u = q*sin' and c = q*cos on DVE straight from PSUM; the
    half-swap swap64(u) runs as an SBUF->SBUF DMA partition rotation
    (2 64-partition copies for q+k together); h = c + sw on the
    otherwise-idle Pool engine (SBUF-only operands).
  - scores in BOTH orientations ([q,k] for ACT-accumulated row sums,
    [k,q] for the AV matmul); exp on ACT; no max-subtraction.
  - V transposed seq-major via PE-transpose (bf16: 1 cycle/row).
  - softmax normalization (1/rowsum) folded into the PSUM->SBUF copy of
    the output projection as a per-partition scale (ACT/DVE split).
"""

import math

import numpy as np
import ml_dtypes

import concourse.bass as bass
import concourse.mybir as mybir
import concourse.tile as tile
from concourse.bass_utils import run_bass_kernel_spmd

B, S, DIM, HD = 256, 200, 1024, 128
BASE = 10000.0
N_CORES = 8
BS = B // N_CORES      # 32 batches per core
BB = 2                 # batches per block
TB = BB * S            # 400 tokens per block
NBLK = BS // BB        # 16 blocks per core
NDC = DIM // 128       # 8 contraction chunks
F32 = mybir.dt.float32
BF16 = mybir.dt.bfloat16
EXP_SCALE = 1.0 / math.sqrt(HD)
# q-position (= output row) chunks within one batch: [0:128], [128:200]
QCH = [(0, 128), (128, S - 128)]


class _TileContextSplitDrain(tile.TileContext):
    """Workaround: this walrus build rejects >2 sem-wait commands on the
    kernel-tail Drain. Emit each needed wait as its own instruction first."""

    def _drain_and_barrier(self, tick_clock, wait_clock):
        nc = self.nc
        fake = mybir.InstNoOp(
            name=nc.get_next_instruction_name(), ins=[], outs=[],
            engine=mybir.EngineType.SP,
        )
        wait_clock.add_sem_waits(
            fake, tile.ScopedClock({None: tick_clock.global_clock})
        )
        waits = list(fake.sync_info.on_wait) if fake.sync_info is not None else []
        assert self.sems is not None
        handles = {h.name: h for h in self.sems.allocated().values()}
        for w in waits:
            nc.sync.wait_ge(handles[w.ant_name], w.wait_value)
        nc.sync.drain()
        nc.all_engine_barrier()
        popped = nc._tile_sem_poison_stack.pop()
        assert popped is self._sem_poison
        nc.clear_and_free_semaphores(list(self.sems.allocated().values()))
        nc.all_engine_barrier()


def _split_excess_waits(nc):
    """This walrus build accepts 1 sem-wait per instruction (2 on
    EventSemaphore). Tile may attach more; hoist the excess onto standalone
    EventSemaphore instructions right before the owner (same engine, so
    in-order issue preserves the wait semantics)."""
    n = 0
    for b in nc.m.functions[0].blocks:
        insts = b.instructions
        out = []
        for i in insts:
            si = i.sync_info
            if si is not None and len(si.on_wait) > 1:
                keep = 2 if isinstance(i, mybir.InstEventSemaphore) else 1
                waits = list(si.on_wait)
                for w in waits[:-keep] if keep < len(waits) else []:
                    n += 1
                    out.append(mybir.InstEventSemaphore(
                        name=f"{i.name}-evw{n}", ins=[], outs=[],
                        engine=i.engine,
                        sync_info=mybir.SyncInfo(on_wait=[w], on_update=[]),
                    ))
                i.sync_info = mybir.SyncInfo(
                    on_wait=waits[-keep:], on_update=list(si.on_update)
                )
            out.append(i)
        b.instructions = out
    return n


def _build_nc():
    nc = bass.Bass("TRN2", target_bir_lowering=False, debug=False)

    xt = nc.dram_tensor("xt", [NBLK, 128, NDC * TB], BF16, kind="ExternalInput").ap()
    wqt = nc.dram_tensor("wqt", [128, NDC * HD], BF16, kind="ExternalInput").ap()
    wkt = nc.dram_tensor("wkt", [128, NDC * HD], BF16, kind="ExternalInput").ap()
    wvt = nc.dram_tensor("wvt", [128, NDC * HD], BF16, kind="ExternalInput").ap()
    wot = nc.dram_tensor("wot", [HD, DIM], BF16, kind="ExternalInput").ap()
    cosf = nc.dram_tensor("cosf", [128, TB], F32, kind="ExternalInput").ap()
    s2f = nc.dram_tensor("s2f", [128, TB], F32, kind="ExternalInput").ap()
    ident = nc.dram_tensor("ident", [128, 128], BF16, kind="ExternalInput").ap()
    out = nc.dram_tensor("out", [BS, S, DIM], BF16, kind="ExternalOutput").ap()

    with _TileContextSplitDrain(nc) as tc:
        with (
            tc.tile_pool(name="singles", bufs=1) as singles,
            tc.tile_pool(name="xt", bufs=3) as xt_pool,
            tc.tile_pool(name="qkv_ps", bufs=2, space="PSUM") as qkv_ps,
            tc.tile_pool(name="attn_ps", bufs=4, space="PSUM") as attn_ps,
            tc.tile_pool(name="out_ps", bufs=2, space="PSUM") as out_ps,
            tc.tile_pool(name="ropetmp", bufs=2) as ropetmp,
            tc.tile_pool(name="heads", bufs=3) as heads,
            tc.tile_pool(name="attn_sb", bufs=4) as attn_sb_pool,
            tc.tile_pool(name="stats", bufs=8) as stats,
            tc.tile_pool(name="outsb", bufs=3) as outsb,
        ):
            # ---- one-time loads (wq + first x first: they gate the first
            # projection matmuls; everything else can land behind) ----
            w_sb = {}
            t = singles.tile([128, NDC * HD], BF16, name="wq", tag="wq")
            nc.sync.dma_start(out=t, in_=wqt)
            w_sb["wq"] = t
            xt0_sb = xt_pool.tile([128, NDC * TB], BF16, name="xt", tag="xt")
            nc.sync.dma_start(out=xt0_sb, in_=xt[0])
            for name, src in (("wk", wkt), ("wv", wvt)):
                t = singles.tile([128, NDC * HD], BF16, name=name, tag=name)
                nc.sync.dma_start(out=t, in_=src)
                w_sb[name] = t
            cos_sb = singles.tile([128, TB], F32, name="cosf", tag="cosf")
            nc.sync.dma_start(out=cos_sb, in_=cosf)
            s2_sb = singles.tile([128, TB], F32, name="s2f", tag="s2f")
            nc.sync.dma_start(out=s2_sb, in_=s2f)
            id_sb = singles.tile([128, 128], BF16, name="ident", tag="ident")
            nc.sync.dma_start(out=id_sb, in_=ident)
            wot_sb = singles.tile([HD, DIM], BF16, name="wot", tag="wot")
            nc.sync.dma_start(out=wot_sb, in_=wot)

            # Two-stage software pipeline: stage A(blk) = load + QKV
            # projections + RoPE (whose DVE->DMA-swap->Pool chain has ~6us
            # of latency); stage B(blk) = attention + output projection.
            # Emitting A(blk+1) between A(blk) and B(blk) gives every
            # engine's in-order stream enough independent work to hide the
            # RoPE chain.
            state = {}

            def stage_a(blk):
                if blk == 0:
                    xt_sb = xt0_sb
                else:
                    xt_sb = xt_pool.tile([128, NDC * TB], BF16, name="xt",
                                         tag="xt")
                    nc.sync.dma_start(out=xt_sb, in_=xt[blk])

                # ---- QKV projections (head-major [128h, TB]) ----
                def proj(wname):
                    ps = qkv_ps.tile([128, TB], F32, name="proj_ps", tag="proj_ps")
                    for dc in range(NDC):
                        nc.tensor.matmul(
                            ps,
                            lhsT=w_sb[wname][:, dc * HD:(dc + 1) * HD],
                            rhs=xt_sb[:, dc * TB:(dc + 1) * TB],
                            start=(dc == 0),
                            stop=(dc == NDC - 1),
                        )
                    return ps

                q_ps = proj("wq")
                k_ps = proj("wk")
                v_ps = proj("wv")

                # ---- RoPE (de-interleaved rotate-half form) ----
                # rotated = q*cosf + swap64(q)*sinf, where sinf is
                # sign-folded. With s2 = swap64(sinf): swap64(q)*sinf =
                # swap64(q*s2), and the partition half-swap runs as an
                # SBUF->SBUF DMA (q and k in one tile -> 2 DMAs total).
                u2 = ropetmp.tile([128, 2 * TB], BF16, name="u2", tag="u2")
                c2 = ropetmp.tile([128, 2 * TB], BF16, name="c2", tag="c2")
                sw2 = ropetmp.tile([128, 2 * TB], BF16, name="sw2", tag="sw2")
                # u-muls first: the swap DMA (long latency) waits only on
                # these two, c-muls fill DVE behind it.
                nc.vector.tensor_mul(u2[:, 0:TB], q_ps, s2_sb)
                nc.vector.tensor_mul(u2[:, TB:2 * TB], k_ps, s2_sb)
                nc.sync.dma_start(out=sw2[0:64, :], in_=u2[64:128, :])
                nc.sync.dma_start(out=sw2[64:128, :], in_=u2[0:64, :])
                nc.vector.tensor_mul(c2[:, 0:TB], q_ps, cos_sb)
                nc.vector.tensor_mul(c2[:, TB:2 * TB], k_ps, cos_sb)
                q_h = heads.tile([128, TB], BF16, name="q_h", tag="q_h")
                k_h = heads.tile([128, TB], BF16, name="k_h", tag="k_h")
                for col, h in ((0, q_h), (TB, k_h)):
                    nc.gpsimd.tensor_add(h, c2[:, col:col + TB],
                                         sw2[:, col:col + TB])
                v_h = heads.tile([128, TB], BF16, name="v_h", tag="v_h")
                nc.scalar.copy(v_h, v_ps)
                state[blk] = (q_h, k_h, v_h)

            def stage_b(blk):
                q_h, k_h, v_h = state.pop(blk)

                # ---- V -> seq-major via PE transpose, per batch ----
                vt_sbs = []
                for i in range(BB):
                    vt_ps = attn_ps.tile([128, 256], BF16, name="vt_ps",
                                         tag="attn_ps")
                    nc.tensor.transpose(
                        vt_ps[0:128, 0:128],
                        v_h[:, i * S: i * S + 128], id_sb,
                    )
                    nc.tensor.transpose(
                        vt_ps[0:72, 128:256],
                        v_h[:, i * S + 128: (i + 1) * S], id_sb,
                    )
                    vt_sb = attn_sb_pool.tile([128, 256], BF16, name="vt_sb",
                                              tag="vt_sb")
                    nc.vector.tensor_copy(vt_sb[0:128, 0:128],
                                          vt_ps[0:128, 0:128])
                    nc.vector.tensor_copy(vt_sb[0:72, 128:256],
                                          vt_ps[0:72, 128:256])
                    vt_sbs.append(vt_sb)

                # ---- scores [q,k] -> exp row-sums via ACT accum_out ----
                recips = {}
                for qc, (q0, qsz) in enumerate(QCH):
                    sp = attn_ps.tile([128, TB], F32, name="s_ps",
                                      tag="attn_ps")
                    for i in range(BB):
                        nc.tensor.matmul(
                            sp[0:qsz, i * S:(i + 1) * S],
                            lhsT=q_h[:, i * S + q0: i * S + q0 + qsz],
                            rhs=k_h[:, i * S:(i + 1) * S],
                            start=True, stop=True,
                        )
                    scratch = attn_sb_pool.tile([128, TB], BF16,
                                                name="exp_scr", tag="exp_scr")
                    sums = stats.tile([128, BB], F32, name="sums", tag="sums")
                    for i in range(BB):
                        nc.scalar.activation(
                            out=scratch[0:qsz, i * S:(i + 1) * S],
                            in_=sp[0:qsz, i * S:(i + 1) * S],
                            func=mybir.ActivationFunctionType.Exp,
                            scale=EXP_SCALE,
                            accum_out=sums[0:qsz, i:i + 1],
                        )
                    rec = stats.tile([128, BB], F32, name="recip", tag="recip")
                    nc.vector.reciprocal(rec[0:qsz, :], sums[0:qsz, :])
                    recips[qc] = rec

                # ---- scores [k,q] -> exp'd (unnormalized probs for AV) ----
                est = []
                for kc, (k0, ksz) in enumerate(QCH):
                    stp = attn_ps.tile([128, TB], F32, name="st_ps",
                                       tag="attn_ps")
                    for i in range(BB):
                        nc.tensor.matmul(
                            stp[0:ksz, i * S:(i + 1) * S],
                            lhsT=k_h[:, i * S + k0: i * S + k0 + ksz],
                            rhs=q_h[:, i * S:(i + 1) * S],
                            start=True, stop=True,
                        )
                    e = attn_sb_pool.tile([128, TB], BF16, name="exp_st",
                                          tag="exp_st")
                    nc.scalar.activation(
                        out=e[0:ksz, :], in_=stp[0:ksz, :],
                        func=mybir.ActivationFunctionType.Exp,
                        scale=EXP_SCALE,
                    )
                    est.append(e)

                # ---- AV: attn_head[h, t] (unnormalized) ----
                av_ps = attn_ps.tile([128, TB], F32, name="av_ps",
                                     tag="attn_ps")
                for i in range(BB):
                    nc.tensor.matmul(
                        av_ps[:, i * S:(i + 1) * S],
                        lhsT=vt_sbs[i][0:128, 0:128],
                        rhs=est[0][0:128, i * S:(i + 1) * S],
                        start=True, stop=False,
                    )
                    nc.tensor.matmul(
                        av_ps[:, i * S:(i + 1) * S],
                        lhsT=vt_sbs[i][0:72, 128:256],
                        rhs=est[1][0:72, i * S:(i + 1) * S],
                        start=False, stop=True,
                    )
                av_sb = attn_sb_pool.tile([128, TB], BF16, name="av_sb",
                                          tag="av_sb")
                nc.vector.tensor_copy(av_sb, av_ps)

                # ---- output projection + normalization + store ----
                for i in range(BB):
                    for tc_i, (t0, tsz) in enumerate(QCH):
                        osb = outsb.tile([128, DIM], BF16,
                                         name=f"osb{i}_{tc_i}",
                                         tag=f"osb{i}_{tc_i}")
                        for dc in range(2):
                            ops = out_ps.tile([128, 512], F32, name="out_ps",
                                              tag="out_ps")
                            nc.tensor.matmul(
                                ops[0:tsz, :],
                                lhsT=av_sb[:, i * S + t0: i * S + t0 + tsz],
                                rhs=wot_sb[:, dc * 512:(dc + 1) * 512],
                                start=True, stop=True,
                            )
                            dst = osb[0:tsz, dc * 512:(dc + 1) * 512]
                            if dc == 0:
                                nc.scalar.mul(dst, ops[0:tsz, :],
                                              recips[tc_i][0:tsz, i:i + 1])
                            else:
                                nc.vector.tensor_scalar_mul(
                                    dst, ops[0:tsz, :],
                                    recips[tc_i][0:tsz, i:i + 1])
                        nc.sync.dma_start(
                            out=out[blk * BB + i, t0:t0 + tsz, :],
                            in_=osb[0:tsz, :],
                        )

            for blk in range(NBLK + 1):
                if blk < NBLK:
                    stage_a(blk)
                if blk >= 1:
                    stage_b(blk - 1)
    _split_excess_waits(nc)
    return nc


_NC_CACHE = {}


def _get_nc():
    if "nc" not in _NC_CACHE:
        _NC_CACHE["nc"] = _build_nc()
    return _NC_CACHE["nc"]


def _host_prep(x, wq, wk, wv, wo):
    """Shared (non-x) device inputs + per-core x^T shards (all bf16)."""
    perm = np.concatenate([np.arange(0, HD, 2), np.arange(1, HD, 2)])
    # weight layout [p, dc*HD + h]: row d of w.T at (p=d%128, dc=d//128)
    def wlay(w):
        return np.ascontiguousarray(
            w.T.reshape(NDC, 128, HD).transpose(1, 0, 2).reshape(128, NDC * HD)
        ).astype(ml_dtypes.bfloat16)

    wqt = wlay(wq[perm])
    wkt = wlay(wk[perm])
    wvt = wlay(wv)
    wot = np.ascontiguousarray(wo.T).astype(ml_dtypes.bfloat16)

    inv_freq = 1.0 / BASE ** (np.arange(0, HD, 2, dtype=np.float64) / HD)
    ang = np.zeros((S, HD // 2), np.float64)
    ang[1:] = np.arange(S - 1, dtype=np.float64)[:, None] * inv_freq[None, :]
    cos_t = np.cos(ang).T.astype(np.float32)   # [64, S]
    sin_t = np.sin(ang).T.astype(np.float32)
    cosf = np.tile(np.concatenate([cos_t, cos_t], axis=0), (1, BB))  # [128, TB]
    # sign-folded: rotated = q*cosf + swap64(q)*sinf in one add
    sinf = np.tile(np.concatenate([-sin_t, sin_t], axis=0), (1, BB))
    # s2 = swap64(sinf) so that swap64(q*s2) == swap64(q)*sinf
    s2f = np.roll(sinf, -64, axis=0)
    ident = np.eye(128, dtype=ml_dtypes.bfloat16)

    shared = {
        "wqt": wqt, "wkt": wkt, "wvt": wvt, "wot": wot,
        "cosf": np.ascontiguousarray(cosf),
        "s2f": np.ascontiguousarray(s2f),
        "ident": ident,
    }
    xts = []
    for c in range(N_CORES):
        xc = x[c * BS:(c + 1) * BS].reshape(NBLK, TB, NDC, 128)
        # [blk, p, dc*TB + t]: one contiguous 6.4KB run per partition per DMA
        xtc = np.ascontiguousarray(
            xc.transpose(0, 3, 2, 1).reshape(NBLK, 128, NDC * TB)
        ).astype(ml_dtypes.bfloat16)
        xts.append(xtc)
    return shared, xts


def kernel(x, wq, wk, wv, wo):
    x = np.asarray(x, np.float32)
    wq = np.asarray(wq, np.float32)
    wk = np.asarray(wk, np.float32)
    wv = np.asarray(wv, np.float32)
    wo = np.asarray(wo, np.float32)

    shared, xts = _host_prep(x, wq, wk, wv, wo)
    in_maps = [dict(shared, xt=xts[c]) for c in range(N_CORES)]
    nc = _get_nc()
    res = run_bass_kernel_spmd(nc, in_maps, list(range(N_CORES)))
    return np.concatenate(
        [np.asarray(res.results[c]["out"]).astype(np.float32)
         for c in range(N_CORES)], axis=0
    ).reshape(B, S, DIM)
